# revision 49
# baseline (speedup 1.0000x reference)
"""HGNN metapath GRU + edge-softmax message passing on 8 TRN2 NeuronCores.

Strategy (self-contained, full inputs in / full output out):
 - Edges are sharded by DESTINATION NODE RANGE: core c owns nodes
   [c*2500, (c+1)*2500) and every edge whose dst lands there (host sorts
   edges by dst).  All segment ops (softmax sum + message scatter) are then
   core-local: zero collectives.
 - The final two linear layers are folded through the segment-sum:
   out[n] = sum_h Q[n,h,:]/S[n,h] + bc,  where per-edge
   q[e,(h,i)] = exp(lrelu(a))[e,h] * (eft[e] @ BA)[.,(h,i)] is scattered
   with one-hot matmuls (one-hot matrices precomputed on host from indices).
 - GRU runs feature-major (gate dims on partitions, edges on the free dim);
   node embeddings are gathered feature-major via dma_gather(transpose=True)
   from a bf16 node-major DRAM table computed on-device.
 - GRU W_hh matmuls run in fp8e4m3 DoubleRow mode (2 contraction subtiles
   per instruction); the hidden state lives in fp8 (produced on the Pool
   engine), gate activations are paired into multi-bank PSUM tiles (valid
   because all GRU biases are zero here), and the 4 GRU steps of
   consecutive edge tiles are software-pipelined 4 deep so every
   engine stays busy despite the step-to-step dependency chain.
 - Final hidden states spill to DRAM and are re-loaded chunk-wise by the
   attention/scatter phase.
"""

import sys
import numpy as np

sys.path.insert(0, "/opt/trn_rl_repo")

import ml_dtypes  # noqa: E402

N_NODES = 20000
N_CORES = 8
NPC = N_NODES // N_CORES          # 2500 nodes per core
NODE_CHUNKS = (NPC + 127) // 128  # 20
WALK = 4
FEAT = 256
HID = 64
NH = 8
HR = NH * HID                     # 512
G3 = 3 * HR                       # 1536
OUT_DIM = 16
E_TILE = 512
NP_PAD = ((N_NODES + 511) // 512) * 512  # 20480 padded node rows

bf = ml_dtypes.bfloat16
f8 = ml_dtypes.float8_e4m3fn


def _wrap_idx(v):
    """int array [n] -> wrapped int16 [128, n//16] layout for dma_gather."""
    n = v.shape[0]
    assert n % 16 == 0
    w = v.reshape(n // 16, 16).T.astype(np.int16)      # [16, n//16]
    return np.tile(w, (8, 1))                           # [128, n//16]


def _host_prep(x, W_mlp, b_mlp, W_ih, W_hh, b_ih, b_hh, attn, W_emb, b_emb,
               W_last, b_last, edge_metapath_indices):
    idx = np.asarray(edge_metapath_indices).astype(np.int64)
    E = idx.shape[0]
    dst = idx[:, -1]
    core = np.clip(dst // NPC, 0, N_CORES - 1)

    per_core_eids = []
    for c in range(N_CORES):
        sel = np.nonzero(core == c)[0]
        order = np.argsort(dst[sel], kind="stable")
        per_core_eids.append(sel[order])
    counts = [len(e) for e in per_core_eids]
    E_pad = max(512, ((max(counts) + E_TILE - 1) // E_TILE) * E_TILE)
    n_tiles = E_pad // E_TILE
    n_ech = E_pad // 128

    # per-core sorted/padded indices + local dst
    sidx = np.zeros((N_CORES, E_pad, WALK), np.int64)
    ldst = np.full((N_CORES, E_pad), -1000, np.int64)
    for c in range(N_CORES):
        e = per_core_eids[c]
        sidx[c, :len(e)] = idx[e]
        ldst[c, :len(e)] = dst[e] - c * NPC

    # gather indices: per tile, 4*E_TILE idxs (step-major)
    gidx = np.zeros((N_CORES, n_tiles, 128, (WALK * E_TILE) // 16), np.int16)
    for c in range(N_CORES):
        for t in range(n_tiles):
            v = sidx[c, t * E_TILE:(t + 1) * E_TILE, :].T.reshape(-1)  # [4*E_TILE]
            gidx[c, t] = _wrap_idx(v)

    # shared scatter schedule: union over cores of node-chunks touched per edge-chunk
    pairs = []            # list of (k, j)
    pair_of = {}
    for k in range(n_ech):
        js = set()
        for c in range(N_CORES):
            d = ldst[c, k * 128:(k + 1) * 128]
            js |= set((d[d >= 0] // 128).tolist())
        if js:
            for j in range(min(js), max(js) + 1):
                pair_of[(k, j)] = len(pairs)
                pairs.append((k, j))
    first_k, last_k = {}, {}
    for (k, j) in pairs:
        first_k.setdefault(j, k)
        last_k[j] = k
    # max concurrently-active accumulators
    active, max_active = set(), 0
    for k in range(n_ech):
        for (kk, j) in pairs:
            if kk == k:
                if first_k[j] == k:
                    active.add(j)
        max_active = max(max_active, len(active))
        for j in list(active):
            if last_k[j] == k:
                active.discard(j)
    n_pairs = len(pairs)

    oneh = np.zeros((N_CORES, max(n_pairs, 1), 128, 128), bf)
    m_ids = np.arange(128)
    for c in range(N_CORES):
        for p, (k, j) in enumerate(pairs):
            d = ldst[c, k * 128:(k + 1) * 128]
            oneh[c, p] = (d[:, None] == (j * 128 + m_ids)[None, :]).astype(bf)

    # weights
    Wc = (np.asarray(W_last, np.float32) @ np.asarray(W_emb, np.float32))  # [16, 512]
    BA = np.zeros((HR, 136), np.float32)
    attn = np.asarray(attn, np.float32)
    for h in range(NH):
        BA[h * HID:(h + 1) * HID, h * OUT_DIM:(h + 1) * OUT_DIM] = \
            Wc[:, h * HID:(h + 1) * HID].T
        BA[h * HID:(h + 1) * HID, 128 + h] = attn[0, h, :]
    ba_p = BA.reshape(4, 128, 136).transpose(1, 0, 2).reshape(128, 4 * 136).astype(bf)

    W_hhT = np.asarray(W_hh, np.float32).T                       # [512, 1536]
    # fp8 DoubleRow layout: whh8[p, k, m, mm] = W_hhT[k*128+p, m*128+mm]
    whh8 = np.zeros((128, 4, 12, 128), np.float32)
    for k in range(4):
        whh8[:, k] = W_hhT[k * 128:(k + 1) * 128, :].reshape(128, 12, 128)
    whh8 = whh8.reshape(128, 4 * 12 * 128).astype(f8)
    wih_p = np.asarray(W_ih, np.float32).T.astype(bf)            # [64, 1536]
    wmlp_p = np.asarray(W_mlp, np.float32).T.astype(bf)          # [256, 64]

    b_ih = np.asarray(b_ih, np.float32)
    b_hh = np.asarray(b_hh, np.float32)
    assert not np.any(b_ih != 0.0) and not np.any(b_hh != 0.0), \
        "fast path requires zero GRU biases"

    b_mlp = np.asarray(b_mlp, np.float32)
    has_bmlp = bool(np.any(b_mlp != 0.0))
    bmlp_row = np.tile(b_mlp[None, :], (128, 1)).astype(np.float32)  # [128, 64]

    bc_vec = (np.asarray(b_emb, np.float32) @ np.asarray(W_last, np.float32).T
              + np.asarray(b_last, np.float32))                  # [16]
    bc_t = np.tile(bc_vec[None, :], (128, 1)).astype(np.float32)

    # x transposed (feature-major) so phase 1 needs no on-device transposes
    x_pad = np.zeros((FEAT, NP_PAD), bf)
    x_pad[:, :N_NODES] = np.asarray(x, np.float32).astype(bf).T

    plan = dict(E_pad=E_pad, n_tiles=n_tiles, n_ech=n_ech, pairs=pairs,
                pair_of=pair_of, first_k=first_k, last_k=last_k,
                max_active=max_active, n_pairs=n_pairs,
                has_bmlp=has_bmlp,
                flushed=set(last_k.keys()), bc_vec=bc_vec)
    wmlp_pk = wmlp_p.reshape(2, 128, HID).transpose(1, 0, 2).reshape(128, 2 * HID)
    shared = dict(x=x_pad, wmlp=np.ascontiguousarray(wmlp_pk), wih=wih_p,
                  whh8=whh8, ba=ba_p, bmlp=bmlp_row, bc=bc_t)
    percore = dict(gidx=gidx, oneh=oneh)
    return plan, shared, percore


def _build(plan, phases=3):
    from contextlib import ExitStack
    import concourse.bass as bass
    import concourse.tile as tile
    from concourse import bacc, mybir

    f32 = mybir.dt.float32
    bf16 = mybir.dt.bfloat16
    fp8 = mybir.dt.float8e4
    i16 = mybir.dt.int16
    AF = mybir.ActivationFunctionType
    OP = mybir.AluOpType
    DR = mybir.MatmulPerfMode.DoubleRow
    P = 128

    E_pad, n_tiles, n_ech = plan["E_pad"], plan["n_tiles"], plan["n_ech"]
    pairs, pair_of = plan["pairs"], plan["pair_of"]
    first_k, last_k = plan["first_k"], plan["last_k"]
    has_bmlp = plan["has_bmlp"]
    acc_bufs = min(6, max(2, plan["max_active"] + 1))

    nc = bacc.Bacc("TRN2", target_bir_lowering=False, debug=False)

    x_d = nc.dram_tensor("x", [FEAT, NP_PAD], bf16, kind="ExternalInput")
    wmlp_d = nc.dram_tensor("wmlp", [P, 2 * HID], bf16, kind="ExternalInput")
    wih_d = nc.dram_tensor("wih", [HID, G3], bf16, kind="ExternalInput")
    whh8_d = nc.dram_tensor("whh8", [P, 4 * 12 * P], fp8, kind="ExternalInput")
    ba_d = nc.dram_tensor("ba", [P, 4 * 136], bf16, kind="ExternalInput")
    bmlp_d = nc.dram_tensor("bmlp", [P, HID], f32, kind="ExternalInput")
    bc_d = nc.dram_tensor("bc", [P, OUT_DIM], f32, kind="ExternalInput")
    gidx_d = nc.dram_tensor("gidx", [n_tiles, P, (WALK * E_TILE) // 16], i16,
                            kind="ExternalInput")
    oneh_d = nc.dram_tensor("oneh", [max(plan["n_pairs"], 1), P, P], bf16,
                            kind="ExternalInput")
    out_d = nc.dram_tensor("out", [NODE_CHUNKS * P, OUT_DIM], f32,
                           kind="ExternalOutput")
    etab_d = nc.dram_tensor("etab", [NP_PAD, P], bf16, kind="Internal")
    hT_d = nc.dram_tensor("hT", [P, 4, E_pad], bf16, kind="Internal")

    from concourse.masks import make_identity

    with tile.TileContext(nc) as tc, ExitStack() as ctx:
        wpool = ctx.enter_context(tc.tile_pool(name="w", bufs=1))
        wih_sb = wpool.tile([HID, G3], bf16, tag="wih")
        nc.sync.dma_start(wih_sb[:], wih_d[:])
        whh8_sb = wpool.tile([P, 4, 12, P], fp8, tag="whh8")
        nc.sync.dma_start(whh8_sb[:].rearrange("p a b c -> p (a b c)"), whh8_d[:])
        ba_sb = wpool.tile([P, 4 * 136], bf16, tag="ba")
        nc.sync.dma_start(ba_sb[:], ba_d[:])
        bmlp_sb = wpool.tile([P, HID], f32, tag="bmlp")
        nc.sync.dma_start(bmlp_sb[:], bmlp_d[:])
        bc_sb = wpool.tile([P, OUT_DIM], f32, tag="bc")
        nc.sync.dma_start(bc_sb[:], bc_d[:])
        wm_sb = wpool.tile([P, 2 * HID], bf16, tag="wm")  # packed k-chunks of W_mlp.T
        nc.sync.dma_start(wm_sb[:], wmlp_d[:])

        # ---------------- phase 1: embedding table ----------------
        # x arrives feature-major (xT), so the emb matmul needs no on-device
        # transposes; 4 node-chunks (512 rows) per DMA for low SP load.
        with tc.tile_pool(name="e_sb", bufs=3) as epool, \
             tc.tile_pool(name="e_ps", bufs=4, space="PSUM") as epsum:
            n_blk = NP_PAD // (4 * P)  # 40
            for blk in range(n_blk):
                n0 = blk * 4 * P
                xin = epool.tile([P, 2, 4 * P], bf16, tag="xin")
                nc.sync.dma_start(
                    xin[:], x_d[:, n0:n0 + 4 * P]
                        .rearrange("(k p) n -> p k n", k=2))
                esb = epool.tile([P, 4, P], bf16, tag="esb")
                nc.vector.memset(esb[:, :, HID:P], 0)
                for cc in range(4):
                    ep = epsum.tile([P, HID], f32, tag="ep", space="PSUM")
                    nc.tensor.matmul(ep[:], xin[:, 0, cc * P:(cc + 1) * P],
                                     wm_sb[:, 0:HID], start=True, stop=False)
                    nc.tensor.matmul(ep[:], xin[:, 1, cc * P:(cc + 1) * P],
                                     wm_sb[:, HID:2 * HID],
                                     start=False, stop=True)
                    if has_bmlp:
                        nc.vector.tensor_tensor(esb[:, cc, 0:HID], ep[:],
                                                bmlp_sb[:], OP.add)
                    elif cc % 2 == 0:
                        nc.vector.tensor_copy(esb[:, cc, 0:HID], ep[:])
                    else:
                        nc.scalar.copy(esb[:, cc, 0:HID], ep[:])
                nc.sync.dma_start(
                    etab_d[n0:n0 + 4 * P, :]
                        .rearrange("(c p) f -> p c f", c=4), esb[:])

        # ---------------- phase 2: GRU over edge tiles (sw-pipelined) -------
        NIDX = WALK * E_TILE
        if phases < 2:
            n_tiles_run = 0
        else:
            n_tiles_run = n_tiles
        DEPTH = 5   # tiles in flight; each owns gat + h8 + hb state
        PRE = 2     # gathers issued ahead
        with tc.tile_pool(name="g_idx", bufs=3) as ipool, \
             tc.tile_pool(name="g_gat", bufs=DEPTH + PRE + 1) as gpool, \
             tc.tile_pool(name="g_zn", bufs=3) as znpool, \
             tc.tile_pool(name="g_h8", bufs=DEPTH + 2) as h8pool, \
             tc.tile_pool(name="g_hb", bufs=DEPTH + 2) as hbpool, \
             tc.tile_pool(name="g_ho", bufs=3) as hopool, \
             tc.tile_pool(name="g_tmp", bufs=2) as tpool, \
             tc.tile_pool(name="g_rhn", bufs=3) as rhnpool, \
             tc.tile_pool(name="g_pp", bufs=4, space="PSUM") as pppsum:

            def wih_s(m):
                return wih_sb[:, m * P:(m + 1) * P]

            state = {}   # tile idx -> dict(gat=, h8=, hb=)

            def emit_gather(t):
                idxt = ipool.tile([P, NIDX // 16], i16, tag="idx")
                nc.sync.dma_start(idxt[:], gidx_d[t])
                gat = gpool.tile([P, 1, NIDX], bf16, tag="gat")
                nc.gpsimd.dma_gather(gat[:], etab_d[:], idxt[:], NIDX, NIDX, P,
                                     transpose=True, single_packet=False)
                state[t] = dict(gat=gat)

            def x_s(t, s):
                return state[t]["gat"][0:HID, 0, s * E_TILE:(s + 1) * E_TILE]

            def emit_step0(t):
                # z,n gates only (h=0, zero biases); paired psums.
                zt = znpool.tile([P, 4, E_TILE], bf16, tag="zt")
                for zp in range(2):
                    pz = pppsum.tile([P, 2, E_TILE], f32, tag="pp",
                                     space="PSUM")
                    for i in range(2):
                        nc.tensor.matmul(pz[:, i, :], wih_s(4 + 2 * zp + i),
                                         x_s(t, 0), start=True, stop=True)
                    nc.scalar.activation(zt[:, 2 * zp:2 * zp + 2, :], pz[:],
                                         AF.Sigmoid)
                ntp = tpool.tile([P, 4, E_TILE], bf16, tag="nt0")
                for np_i in range(2):
                    pn = pppsum.tile([P, 2, E_TILE], f32, tag="pp",
                                     space="PSUM")
                    for i in range(2):
                        nc.tensor.matmul(pn[:, i, :], wih_s(8 + 2 * np_i + i),
                                         x_s(t, 0), start=True, stop=True)
                    nc.scalar.activation(ntp[:, 2 * np_i:2 * np_i + 2, :],
                                         pn[:], AF.Tanh)
                zn = tpool.tile([P, 4, E_TILE], bf16, tag="zn")
                nc.vector.tensor_tensor(zn[:], zt[:], ntp[:], OP.mult)
                # h0 = n - z*n  (bf16 state + fp8 shadow for matmuls)
                hb = hbpool.tile([P, 4, E_TILE], bf16, tag="hb")
                nc.vector.tensor_tensor(hb[:], ntp[:], zn[:], OP.subtract)
                h8 = h8pool.tile([P, 4, E_TILE], fp8, tag="h8")
                nc.gpsimd.tensor_copy(h8[:], hb[:])
                state[t]["h8"] = h8
                state[t]["hb"] = hb

            def emit_step(t, s):
                final = (s == WALK - 1)
                h8 = state[t]["h8"]
                hb = state[t]["hb"]

                def gate_mms_pair(pp, m_base):
                    for i in range(2):
                        m = m_base + i
                        nc.tensor.matmul(pp[:, i, :], wih_s(m), x_s(t, s),
                                         start=True, stop=False)
                        nc.tensor.matmul(pp[:, i, :], whh8_sb[:, 0:2, m, :],
                                         h8[:, 0:2, :], start=False, stop=False,
                                         perf_mode=DR)
                        nc.tensor.matmul(pp[:, i, :], whh8_sb[:, 2:4, m, :],
                                         h8[:, 2:4, :], start=False, stop=True,
                                         perf_mode=DR)

                # r gates first (earliest consumer), then n (pairs, so rhn
                # frees each psum early), then z (latest consumer)
                rq = tpool.tile([P, 4, E_TILE], bf16, tag="rq")
                for pr_i in range(2):
                    pp = pppsum.tile([P, 2, E_TILE], f32, tag="pp",
                                     space="PSUM")
                    gate_mms_pair(pp, 2 * pr_i)
                    nc.scalar.activation(rq[:, 2 * pr_i:2 * pr_i + 2, :],
                                         pp[:], AF.Sigmoid)
                npre = tpool.tile([P, 4, E_TILE], bf16, tag="npre")
                for cp in range(2):
                    pph = pppsum.tile([P, 2, E_TILE], f32, tag="pp",
                                      space="PSUM")
                    for i in range(2):
                        m = 8 + 2 * cp + i
                        nc.tensor.matmul(pph[:, i, :], whh8_sb[:, 0:2, m, :],
                                         h8[:, 0:2, :], start=True, stop=False,
                                         perf_mode=DR)
                        nc.tensor.matmul(pph[:, i, :], whh8_sb[:, 2:4, m, :],
                                         h8[:, 2:4, :], start=False, stop=True,
                                         perf_mode=DR)
                    ppx = pppsum.tile([P, 2, E_TILE], f32, tag="pp",
                                      space="PSUM")
                    for i in range(2):
                        nc.tensor.matmul(ppx[:, i, :], wih_s(8 + 2 * cp + i),
                                         x_s(t, s), start=True, stop=True)
                    rhn = rhnpool.tile([P, 2, E_TILE], f32, tag="rhn")
                    nc.vector.tensor_tensor(rhn[:], rq[:, 2 * cp:2 * cp + 2, :],
                                            pph[:], OP.mult)
                    nc.vector.tensor_tensor(npre[:, 2 * cp:2 * cp + 2, :],
                                            rhn[:], ppx[:], OP.add)

                zt = znpool.tile([P, 4, E_TILE], bf16, tag="zt")
                for zp in range(2):
                    pz = pppsum.tile([P, 2, E_TILE], f32, tag="pp",
                                     space="PSUM")
                    gate_mms_pair(pz, 4 + 2 * zp)
                    nc.scalar.activation(zt[:, 2 * zp:2 * zp + 2, :], pz[:],
                                         AF.Sigmoid)

                def part2():
                    nt = tpool.tile([P, 4, E_TILE], bf16, tag="nt")
                    nc.scalar.activation(nt[:], npre[:], AF.Tanh)
                    d = tpool.tile([P, 4, E_TILE], bf16, tag="d")
                    nc.vector.tensor_tensor(d[:], hb[:], nt[:], OP.subtract)
                    zd = tpool.tile([P, 4, E_TILE], bf16, tag="zd")
                    nc.vector.tensor_tensor(zd[:], zt[:], d[:], OP.mult)
                    if final:
                        hout = hopool.tile([P, 4, E_TILE], bf16, tag="ho",
                                           name="hout")
                        nc.vector.tensor_tensor(hout[:], nt[:], zd[:], OP.add)
                        # spill hT chunk to DRAM
                        nc.sync.dma_start(
                            hT_d[:, :, t * E_TILE:(t + 1) * E_TILE], hout[:])
                        del state[t]
                    else:
                        hbn = hbpool.tile([P, 4, E_TILE], bf16, tag="hb",
                                          name="hbn")
                        nc.vector.tensor_tensor(hbn[:], nt[:], zd[:], OP.add)
                        h8n = h8pool.tile([P, 4, E_TILE], fp8, tag="h8",
                                          name="h8n")
                        nc.gpsimd.tensor_copy(h8n[:], hbn[:])
                        state[t]["h8"] = h8n
                        state[t]["hb"] = hbn
                return part2

            # rolling schedule: rounds i; tile i starts (gather+step0), tile
            # i-1 runs step1, i-2 step2, i-3 step3.  Each step's tanh+update
            # (part2) is deferred one slot so its late npre never blocks the
            # next slot's activations in the in-order Act queue.
            pending = []
            for i in range(PRE):
                if i < n_tiles_run:
                    emit_gather(i)
            for i in range(n_tiles_run + WALK - 1):
                if i + PRE < n_tiles_run:
                    emit_gather(i + PRE)
                if i < n_tiles_run:
                    emit_step0(i)
                    while pending:
                        pending.pop(0)()
                for s in range(1, WALK):
                    tt = i - s
                    if 0 <= tt < n_tiles_run:
                        p2 = emit_step(tt, s)
                        while pending:
                            pending.pop(0)()
                        pending.append(p2)
            while pending:
                pending.pop(0)()

        # ---------------- phase 3: attention + one-hot scatter ----------------
        with tc.tile_pool(name="s_sb", bufs=2) as spool, \
             tc.tile_pool(name="s_hk", bufs=4) as hkpool, \
             tc.tile_pool(name="s_oh", bufs=4) as ohpool, \
             tc.tile_pool(name="s_pay", bufs=3) as paypool, \
             tc.tile_pool(name="s_ps", bufs=2, space="PSUM") as papsum, \
             tc.tile_pool(name="s_acc", bufs=acc_bufs, space="PSUM") as accpsum:

            chunk_pairs = {}
            if phases != 3:
                pairs = []
            for (k, j) in pairs:
                chunk_pairs.setdefault(k, []).append(j)
            # batch one-hot loads: pids are consecutive; group chunks by 4
            max_npids = 4
            for k0 in range(0, n_ech, 4):
                kjs = [kk for kk in range(k0, min(k0 + 4, n_ech))
                       if chunk_pairs.get(kk)]
                if kjs:
                    lo = pair_of[(kjs[0], chunk_pairs[kjs[0]][0])]
                    hi = pair_of[(kjs[-1], chunk_pairs[kjs[-1]][-1])]
                    max_npids = max(max_npids, hi - lo + 1)
            blk_pid = {}    # k -> base pid of the block's one-hot tile
            acc = {}
            hk4 = None
            oh4 = None
            for k in range(n_ech):
                js = chunk_pairs.get(k)
                if k % 4 == 0:
                    kjs = [kk for kk in range(k, min(k + 4, n_ech))
                           if chunk_pairs.get(kk)]
                    if kjs:
                        hk4 = hkpool.tile([P, 4, 4 * P], bf16, tag="hk",
                                          name="hk4")
                        nc.sync.dma_start(
                            hk4[:], hT_d[:, :, k * P:(k + 4) * P])
                        p_lo = pair_of[(kjs[0], chunk_pairs[kjs[0]][0])]
                        p_hi = pair_of[(kjs[-1], chunk_pairs[kjs[-1]][-1])]
                        npids = p_hi - p_lo + 1
                        oh4 = ohpool.tile([P, max_npids, P], bf16, tag="oh",
                                          name="oh4")
                        nc.sync.dma_start(
                            oh4[:, 0:npids, :],
                            oneh_d[p_lo:p_lo + npids]
                                .rearrange("q p n -> p q n"))
                        for kk in kjs:
                            blk_pid[kk] = p_lo
                if not js:
                    continue
                pa = papsum.tile([P, 136], f32, tag="pa", space="PSUM")
                for kk in range(4):
                    nc.tensor.matmul(pa[:], hk4[:, kk, (k % 4) * P:(k % 4 + 1) * P],
                                     ba_sb[:, kk * 136:(kk + 1) * 136],
                                     start=(kk == 0), stop=(kk == 3))
                e1 = spool.tile([P, NH], f32, tag="e1")
                nc.scalar.activation(e1[:], pa[:, 128:136], AF.Exp)
                e2 = spool.tile([P, NH], f32, tag="e2")
                nc.scalar.activation(e2[:], pa[:, 128:136], AF.Exp, scale=0.01)
                ea = spool.tile([P, NH], f32, tag="ea")
                nc.vector.tensor_tensor(ea[:], e1[:], e2[:], OP.max)
                eae = spool.tile([P, NH, OUT_DIM], f32, tag="eae")
                nc.gpsimd.tensor_copy(eae[:],
                                      ea[:, :, None].to_broadcast([P, NH, OUT_DIM]))
                pay = paypool.tile([P, 136], bf16, tag="pay")
                nc.vector.tensor_tensor(pay[:, 0:128], pa[:, 0:128],
                                        eae[:].rearrange("p a b -> p (a b)"), OP.mult)
                nc.gpsimd.tensor_copy(pay[:, 128:136], ea[:])
                for j in js:
                    pid = pair_of[(k, j)]
                    if first_k[j] == k:
                        acc[j] = accpsum.tile([P, 136], f32, tag="acc",
                                              name=f"acc{j}", space="PSUM")
                    nc.tensor.matmul(acc[j][:], oh4[:, pid - blk_pid[k], :],
                                     pay[:],
                                     start=(first_k[j] == k),
                                     stop=(last_k[j] == k),
                                     skip_group_check=True)
                for j in js:
                    if last_k[j] != k:
                        continue
                    aj = acc.pop(j)
                    sc = spool.tile([P, NH], f32, tag="sc")
                    nc.vector.tensor_scalar(sc[:], aj[:, 128:136], 1e-30, None,
                                            OP.max)
                    rc = spool.tile([P, NH], f32, tag="rc")
                    nc.vector.reciprocal(rc[:], sc[:])
                    rce = spool.tile([P, NH, OUT_DIM], f32, tag="rce")
                    nc.gpsimd.tensor_copy(
                        rce[:], rc[:, :, None].to_broadcast([P, NH, OUT_DIM]))
                    wq = spool.tile([P, P], f32, tag="wq")
                    nc.vector.tensor_tensor(wq[:], aj[:, 0:128],
                                            rce[:].rearrange("p a b -> p (a b)"),
                                            OP.mult)
                    o16 = spool.tile([P, OUT_DIM], f32, tag="o16")
                    nc.vector.reduce_sum(
                        o16[:], wq[:].rearrange("p (h i) -> p i h", h=NH),
                        axis=mybir.AxisListType.X)
                    ob = spool.tile([P, OUT_DIM], f32, tag="ob")
                    nc.vector.tensor_tensor(ob[:], o16[:], bc_sb[:], OP.add)
                    nc.sync.dma_start(out_d[j * P:(j + 1) * P, :], ob[:])

    nc.compile()
    return nc


def kernel(**inputs):
    import os
    from concourse.bass_utils import run_bass_kernel_spmd

    num_nodes = int(inputs.pop("num_nodes", N_NODES))
    assert num_nodes == N_NODES
    plan, shared, percore = _host_prep(**inputs)
    nc = _build(plan)

    in_maps = []
    for c in range(N_CORES):
        m = dict(shared)
        m["gidx"] = np.ascontiguousarray(percore["gidx"][c])
        m["oneh"] = np.ascontiguousarray(percore["oneh"][c])
        in_maps.append(m)

    trace = bool(os.environ.get("KERNEL_TRACE"))
    res = run_bass_kernel_spmd(nc, in_maps, core_ids=list(range(N_CORES)),
                               trace=trace)
    global LAST_EXEC_NS, LAST_RESULTS
    LAST_EXEC_NS = getattr(res, "exec_time_ns", None)
    LAST_RESULTS = res

    full = np.empty((N_NODES, OUT_DIM), np.float32)
    for c in range(N_CORES):
        full[c * NPC:(c + 1) * NPC] = res.results[c]["out"][:NPC]
    # node chunks never flushed on device -> pure-bias rows
    for j in range(NODE_CHUNKS):
        if j not in plan["flushed"]:
            for c in range(N_CORES):
                lo = c * NPC + j * 128
                hi = min(c * NPC + min((j + 1) * 128, NPC), (c + 1) * NPC)
                if lo < hi:
                    full[lo:hi] = plan["bc_vec"][None, :]
    return full
